# revision 22
# baseline (speedup 1.0000x reference)
"""
Trainium2 kernel for nn_CanonicalLinear (dense_mlp).

Reference computation:
    heads[b, n, c] = x @ W[n].T + b[n]          (8 per-head linears)
    out[b, c]      = sum_n heads[b, n, c] * factor[n]

By linearity this collapses to a single linear layer:
    W_eff[c, d] = sum_n factor[n] * W[n, c, d]
    b_eff[c]    = sum_n factor[n] * b[n, c]
    out         = x @ W_eff.T + b_eff

The factor reduction is 0.06% of the matmul FLOPs, so it is folded into
the host-side weight preparation (with the [d, c] transpose and a bf16
cast) and the device kernel is a pure streaming matmul: per core
8.6 GFLOP against 20MB of HBM traffic.

Measured hardware behavior (8 cores active, via in-NEFF repeat loops to
cancel ~0.3-1ms dispatch jitter):
  * DMA cost is dominated by per-descriptor issue (~70-85ns per SBUF
    partition row), nearly independent of row size up to ~16-32KB.  All
    tensors are therefore pre-tiled on the host into [128, huge-row]
    layouts where one DMA moves 2-4MB with 16-32KB descriptors, and
    bf16 payloads ride under an fp32 dtype (bitcast to bf16 at matmul
    use) -- 1-2KB-row bf16-typed DMAs measured 3-5x slower.
  * The PE streams bf16 matmuls at ~2 Grows/s (~1 cyc/row, 80-90% of
    the 2.4GHz peak).  262144 moving rows/core => ~125us measured PE
    floor (pure-SBUF probes); this kernel measures ~142us/iteration
    steady-state with all DMA (20MB/core, ~146GB/s) overlapped.
    Loop order (k-outer vs h-outer), PSUM bank count (6/7/8), bf16
    output stores, and 2MB-vs-4MB x blocks all measure equal at r24
    resolution -- the PE row stream is the binding constraint.
  * fp8 (2x PE rate) fails the 2e-2 gate: quantizing either operand to
    e4m3 alone gives ~3.6% Frobenius error on this D=2048 contraction.

Sharding: DP x TP = 4 x 2 over the 8 cores.  Core r = (p, q) =
divmod(r, TP) handles batch rows [p*BS, (p+1)*BS) and output columns
[q*CS, (q+1)*CS).  bf16 inputs halve wire bytes vs fp32 (x 8MB,
W_effT 4MB, out fp32 8MB per core); rel err ~2.4e-3 vs 2e-2 budget.

Device kernel per core:
  1. weffT slice in a c-half-split tiled layout ([128, nh, dk, cs/nh]);
     the first matmuls wait only for the first half (2MB) and the first
     x block (2MB).  Bias row is DMA'd and PE-broadcast via a K=1
     matmul with a ones column.
  2. per block of BLK=4 batch tiles: one x DMA [128, dk*BLK*128]; per
     128-row tile, [128, 512] PSUM chunks accumulate over dk=16
     contraction chunks (bf16 matmuls, fp32 PSUM); DVE adds bias on
     eviction; per-tile output stores keep the drain tail short.
"""

import numpy as np
import ml_dtypes

P = 128
B, D, C, N = 8192, 2048, 2048, 8
DP, TP = 4, 2                      # data-parallel x tensor-parallel grid
BS, CS = B // DP, C // TP          # per-core batch rows / out cols
NCORES = 8
DK = D // P

BLK = 4                            # batch tiles per x/out block DMA
CH = 512                           # psum chunk cols
OUT_BF16 = False
WIRE = "bf16"
KORDER = True                      # k-outer loop: interleave the nh psum
                                   # accumulation chains (hides the PSUM
                                   # read-modify-write hazard) and reuse
                                   # each stationary tile across chains

_cached_nc = None


def set_grid(dp, tp):
    global DP, TP, BS, CS, _cached_nc
    DP, TP = dp, tp
    BS, CS = B // DP, C // TP
    _cached_nc = None


def _build(bs=None, cs=None, d=D, blk=None, ch=None, out_bf16=None,
           wire=None, korder=None, repeat=1, psum7=False, psum8=False,
           oq=False, xbufs=None, pbufs=None):
    import concourse.bass as bass
    import concourse.mybir as mybir
    import concourse.tile as tile
    from concourse import bacc

    bs = BS if bs is None else bs
    cs = CS if cs is None else cs
    blk = BLK if blk is None else blk
    ch = CH if ch is None else ch
    out_bf16 = OUT_BF16 if out_bf16 is None else out_bf16
    wire = WIRE if wire is None else wire
    korder = KORDER if korder is None else korder

    FP32 = mybir.dt.float32
    BF16 = mybir.dt.bfloat16
    F32R = mybir.dt.float32r
    OUT_DT = BF16 if out_bf16 else FP32
    MMD = BF16 if wire == "bf16" else F32R  # matmul dtype
    esz = 2 if wire == "bf16" else 4        # payload element size
    ew = esz // 2                           # fp32 words per 2 payload elems
    # payload columns per fp32 word
    cpw = 4 // esz

    dk = d // P                    # contraction chunks
    nbt = bs // P                  # batch tiles per core
    nblk = (nbt + blk - 1) // blk

    nc = bacc.Bacc()
    # tiled, partition-major layouts; payload bf16 viewed as fp32 words
    xd = nc.dram_tensor("x", [nblk, P, dk * blk * P // cpw], FP32,
                        kind="ExternalInput")
    wd = nc.dram_tensor("w", [P, dk * cs // cpw], FP32,
                        kind="ExternalInput")
    bd = nc.dram_tensor("b", [1, cs], FP32, kind="ExternalInput")
    od = nc.dram_tensor("out", [nblk, P, blk * cs], OUT_DT,
                        kind="ExternalOutput")

    with tile.TileContext(nc) as tc:
        with (
            tc.tile_pool(name="singles", bufs=1) as singles,
            tc.tile_pool(name="wpool", bufs=1) as wpool,
            tc.tile_pool(name="bpool", bufs=1) as bpool,
            tc.tile_pool(name="xtp",
                         bufs=(xbufs or (3 if esz == 2 else 2))) as xtp,
            tc.tile_pool(name="outp", bufs=6) as outp,
            tc.tile_pool(name="pso", bufs=pbufs or (8 if psum8 else
                         (7 if psum7 else 6)), space="PSUM") as pso,
        ):
          ones1 = singles.tile([1, P], FP32)
          nc.vector.memset(ones1, 1.0)
          for _rep in range(repeat):
            # --- weights: c-half-split layout so the first matmuls only
            # wait for half the weight bytes ------------------------------
            TD = FP32 if esz == 2 else F32R
            nh = max(1, cs // ch)
            cs_h = cs // nh
            wsb = wpool.tile([P, nh, dk, cs_h // cpw], TD, tag="wsb")
            wlen = dk * cs_h // cpw

            def w_dma(hi):
                src_ap = wd[:, hi * wlen:(hi + 1) * wlen]
                if esz == 4:
                    src_ap = src_ap.bitcast(F32R)
                nc.sync.dma_start(wsb[:, hi, :, :], src_ap)

            w_dma(0)

            # --- bias: load row, broadcast to 128 partitions via K=1
            # matmul with a ones column ----------------------------------
            brow = bpool.tile([1, cs], FP32, tag="brow")
            nc.sync.dma_start(brow, bd[:])
            beff = bpool.tile([P, cs], FP32, tag="beff")
            for h in range(0, cs, 512):
                hw_ = min(512, cs - h)
                pw = pso.tile([P, 512], FP32, tag="po", name=f"pw_{h}")
                nc.tensor.matmul(pw[:, :hw_], ones1, brow[:1, h:h + hw_])
                nc.any.tensor_copy(beff[:, h:h + hw_], pw[:, :hw_])

            # --- main loop over BLK-tile blocks -------------------------
            def x_dma(b0, nt):
                xtb = xtp.tile([P, dk, blk * P // cpw], TD, tag="xtb",
                               name=f"xtb_{b0}")
                xsrc = (xd[b0, :, :] if nt == blk
                        else xd[b0, :, :dk * nt * P // cpw])
                if esz == 4:
                    xsrc = xsrc.bitcast(F32R)
                nc.sync.dma_start(
                    xtb[:, :, :] if nt == blk else xtb[:, :, :nt * P // cpw],
                    xsrc)
                return xtb

            # first x block between the two weight halves: matmuls on the
            # first c-half start after ~4MB instead of ~6MB
            xtb0 = x_dma(0, min(blk, nbt))
            for hi in range(1, nh):
                w_dma(hi)

            for b0 in range(nblk):
                nt = min(blk, nbt - b0 * blk)
                xtb = xtb0 if b0 == 0 else x_dma(b0, nt)

                def lhs(u, k):
                    if esz == 2:
                        return xtb[:, k,
                                   u * P // cpw:(u + 1) * P // cpw] \
                            .bitcast(MMD)
                    return xtb[:, k, u * P:(u + 1) * P]

                def rhs(k, h, hw_):
                    hi, off = divmod(h, cs_h)
                    if esz == 2:
                        return wsb[:, hi, k,
                                   off // cpw:(off + hw_) // cpw] \
                            .bitcast(MMD)
                    return wsb[:, hi, k, off:off + hw_]

                for u in range(nt):
                    osb = outp.tile([P, cs], OUT_DT, tag="osb",
                                    name=f"osb_{b0}_{u}")
                    if korder:
                        pos = [pso.tile([P, ch], FP32, tag="po",
                                        name=f"po_{u}_{hi}")
                               for hi in range(nh)]
                        for k in range(dk):
                            for hi in range(nh):
                                h = hi * ch
                                hw_ = min(ch, cs - h)
                                nc.tensor.matmul(
                                    pos[hi][:, :hw_],
                                    lhs(u, k),
                                    rhs(k, h, hw_),
                                    start=(k == 0),
                                    stop=(k == dk - 1),
                                )
                        for hi in range(nh):
                            h = hi * ch
                            hw_ = min(ch, cs - h)
                            nc.vector.tensor_add(osb[:, h:h + hw_],
                                                 pos[hi][:, :hw_],
                                                 beff[:, h:h + hw_])
                    else:
                        for h in range(0, cs, ch):
                            hw_ = min(ch, cs - h)
                            po = pso.tile([P, ch], FP32, tag="po")
                            for k in range(dk):
                                nc.tensor.matmul(
                                    po[:, :hw_],
                                    lhs(u, k),
                                    rhs(k, h, hw_),
                                    start=(k == 0),
                                    stop=(k == dk - 1),
                                )
                            nc.vector.tensor_add(osb[:, h:h + hw_],
                                                 po[:, :hw_],
                                                 beff[:, h:h + hw_])
                    # per-u store keeps the drain tail short; optionally
                    # on the Activation HWDGE queue so the sync queue
                    # only carries loads
                    (nc.scalar if oq else nc.sync).dma_start(
                        od[b0, :, u * cs:(u + 1) * cs], osb)

    nc.finalize()
    return nc


def _get_nc():
    global _cached_nc
    if _cached_nc is None:
        _cached_nc = _build(psum8=True, oq=True, xbufs=4)
    return _cached_nc


def _tile_w(weffT_cs, dk, cs, ch):
    # [D, cs] payload -> [128, nh, dk, cs_h] c-half-major partition-major
    nh = max(1, cs // ch)
    cs_h = cs // nh
    a = np.ascontiguousarray(
        weffT_cs.reshape(dk, P, nh, cs_h).transpose(1, 2, 0, 3))
    return a.reshape(P, dk * cs).view(np.float32)


def _tile_x(xT, dk, blk, nblk, esz):
    # [D, BS] payload -> [nblk, 128, dk, blk*128] -> fp32-word view
    bs = xT.shape[1]
    a = xT.reshape(dk, P, nblk, blk * P).transpose(2, 1, 0, 3)
    a = np.ascontiguousarray(a)
    return a.reshape(nblk, P, dk * blk * P).view(np.float32)


def _shard_inputs(x, W, b, factor):
    wdt = ml_dtypes.bfloat16 if WIRE == "bf16" else np.float32
    esz = 2 if WIRE == "bf16" else 4
    nbt = BS // P
    nblk = (nbt + BLK - 1) // BLK
    # host-side weight prep: factor-reduce, transpose, quantize
    weff = np.einsum("n,ncd->cd", factor, W)          # [C, D] fp32
    weffT = np.ascontiguousarray(weff.T).astype(wdt)  # [D, C]
    beff = (factor @ b).astype(np.float32)            # [C]
    in_maps = []
    xsh = {}
    for p in range(DP):
        xs = x[p * BS:(p + 1) * BS]
        xT = np.ascontiguousarray(xs.T).astype(wdt)
        xsh[p] = _tile_x(xT, DK, BLK, nblk, esz)
    wq = {}
    for q in range(TP):
        wq[q] = _tile_w(
            np.ascontiguousarray(weffT[:, q * CS:(q + 1) * CS]),
            DK, CS, CH)
    for r in range(NCORES):
        p, q = divmod(r, TP)
        in_maps.append({
            "x": xsh[p],
            "w": wq[q],
            "b": np.ascontiguousarray(beff[None, q * CS:(q + 1) * CS]),
        })
    return in_maps


def _assemble(res_out_list):
    """res_out_list[r] = out array [nblk, 128, blk*cs] -> full [B, C]."""
    nbt = BS // P
    nblk = (nbt + BLK - 1) // BLK
    out = np.empty((B, C), dtype=np.float32)
    for r in range(NCORES):
        p, q = divmod(r, TP)
        oc = np.asarray(res_out_list[r], dtype=np.float32)
        oc = oc.reshape(nblk, P, BLK, CS).transpose(0, 2, 1, 3)
        oc = oc.reshape(BS, CS)
        out[p * BS:(p + 1) * BS, q * CS:(q + 1) * CS] = oc
    return out


def kernel(x, W, b, factor, _trace=False):
    from concourse.bass_utils import run_bass_kernel_spmd

    x = np.asarray(x, dtype=np.float32)
    W = np.asarray(W, dtype=np.float32)
    b = np.asarray(b, dtype=np.float32)
    factor = np.asarray(factor, dtype=np.float32)

    nc = _get_nc()
    in_maps = _shard_inputs(x, W, b, factor)
    res = run_bass_kernel_spmd(nc, in_maps, list(range(NCORES)),
                               trace=_trace)

    out = _assemble([res.results[r]["out"] for r in range(NCORES)])
    if _trace:
        return out, res
    return out
